# revision 1
# baseline (speedup 1.0000x reference)
"""Trainium2 Bass kernel for nn_MultiHeadAttention_72189810312078.

Computation (per token): qkv = x @ w_qkv.T + b_qkv; per-token attention over
the 16 heads with 16x16 score matrices; out = attn_out @ w_out.T + b_out.

Strategy: data-parallel over 8 NeuronCores (8192 tokens each). Host
pre-transposes x to xT [1024, N] so the channel (contraction) dim lands on
SBUF partitions. Per core, per 256-token superblock:
  1) qkvT projection: 24 feature-chunk matmuls (N=256), K=1024 accumulated in
     PSUM; per-partition bias added in PSUM with one tensor_scalar_add.
  2) PSUM chunks scatter-evicted into attention staging: Q at partitions
     64:128 of T1, K at 64:128 / V at 0:64 of T2 (matmul operands need equal
     base partitions), laid out [d, (group, head, t)].
  3) Attention in groups of 8 tokens ((g,t) packs 16x8=128 partitions):
     scoresT = K.T @ Q per group (K=64 matmul at tile_position row 64);
     exp on ScalarE; multiplicative block-diagonal mask; V8 =
     PE-transpose(V); attnV matmul with a ones column appended to V8 so the
     softmax denominator falls out of the same matmul; normalize with a
     per-partition reciprocal scale on eviction.
  4) attn output PE-transposed back to feature-major, packed into S2
     [128 = (dlt,d), chunk x token]; out-projection against host-permuted
     w_out.T rows (feature 64*(8*dlt+c)+d at S2 row 128c+64*dlt+d); bias
     added from a replicated tile during eviction; result DMA'd row-major.

Dtype mode: projections can run in float32r (fp32 rounded to 11 mantissa
bits, 4x faster on the PE at N>=256) or exact float32. Attention
scores/attnV always accumulate in fp32 PSUM.
"""

import os
import sys
from contextlib import ExitStack

sys.path.insert(0, "/opt/trn_rl_repo")

import numpy as np

import concourse.bass as bass  # noqa: E402
import concourse.bacc as bacc  # noqa: E402
import concourse.tile as tile  # noqa: E402
from concourse import mybir  # noqa: E402
from concourse.bass_utils import run_bass_kernel_spmd  # noqa: E402
from concourse.masks import make_identity  # noqa: E402

F32 = mybir.dt.float32
F32R = mybir.dt.float32r

N_CORES = 8
H, D, C = 16, 64, 1024
SB = 256   # tokens per superblock (projection moving dim)
SS = 128   # tokens per attention sub-stage / out-projection block
NG = SB // 8   # token groups per superblock (32)

USE_F32R = os.environ.get("KMODE", "f32r") == "f32r"
GPS = int(os.environ.get("GPS", "3"))  # bitmask: 1=memset, 2=mask-mul, 4=bias
Exp = mybir.ActivationFunctionType.Exp
Copy = mybir.ActivationFunctionType.Copy


def build(tok, use_f32r=USE_F32R, static_loop=False):
    WD = F32R if use_f32r else F32   # projection operand dtype
    AD = F32R if use_f32r else F32   # attn-out / S2 dtype (out-proj lhsT)

    nc = bacc.Bacc("TRN2", target_bir_lowering=False, debug=False,
                   enable_asserts=True, num_devices=N_CORES)
    xT_d = nc.dram_tensor("xT", [C, tok], WD, kind="ExternalInput").ap()
    wqkvT_d = nc.dram_tensor("wqkvT", [C, 3 * C], WD, kind="ExternalInput").ap()
    woutT_d = nc.dram_tensor("woutT", [C, C], WD, kind="ExternalInput").ap()
    bcols_d = nc.dram_tensor("bcols", [128, 24], F32, kind="ExternalInput").ap()
    borep_d = nc.dram_tensor("borep", [128, C], F32, kind="ExternalInput").ap()
    maskB_d = nc.dram_tensor("maskB", [128, 512], F32, kind="ExternalInput").ap()
    out_d = nc.dram_tensor("out", [tok, C], F32, kind="ExternalOutput").ap()

    with tile.TileContext(nc) as tc, ExitStack() as ctx:
        consts = ctx.enter_context(tc.tile_pool(name="consts", bufs=1))
        xin = ctx.enter_context(tc.tile_pool(name="xin", bufs=2))
        stag = ctx.enter_context(tc.tile_pool(name="stag", bufs=1))
        smx = ctx.enter_context(tc.tile_pool(name="smx", bufs=2))
        s2p = ctx.enter_context(tc.tile_pool(name="s2p", bufs=2))
        outp = ctx.enter_context(tc.tile_pool(name="outp", bufs=2))
        psA = ctx.enter_context(tc.tile_pool(name="psA", bufs=2, space="PSUM"))
        psSp = ctx.enter_context(tc.tile_pool(name="psSp", bufs=2, space="PSUM"))
        psVp = ctx.enter_context(tc.tile_pool(name="psVp", bufs=1, space="PSUM"))
        psC2p = ctx.enter_context(tc.tile_pool(name="psC2p", bufs=1, space="PSUM"))
        psTp = ctx.enter_context(tc.tile_pool(name="psTp", bufs=1, space="PSUM"))
        psOp = ctx.enter_context(tc.tile_pool(name="psOp", bufs=1, space="PSUM"))

        # ---- constants ----
        wq_sb = consts.tile([128, 8, 3 * C], WD)
        nc.sync.dma_start(out=wq_sb, in_=wqkvT_d.rearrange("(ci p) f -> p ci f", p=128))
        wo_sb = consts.tile([128, 8, C], WD)
        nc.sync.dma_start(out=wo_sb, in_=woutT_d.rearrange("(ci p) f -> p ci f", p=128))
        bcols_sb = consts.tile([128, 24], F32)
        nc.sync.dma_start(out=bcols_sb, in_=bcols_d)
        borep_sb = consts.tile([128, C], F32)
        nc.sync.dma_start(out=borep_sb, in_=borep_d)
        maskB_sb = consts.tile([128, 512], F32)
        nc.sync.dma_start(out=maskB_sb, in_=maskB_d)
        idq = consts.tile([128, 128], F32)
        make_identity(nc, idq)
        if AD is F32:
            idr = idq
        else:
            idr = consts.tile([128, 128], AD)
            nc.vector.tensor_copy(idr, idq)

        ecnt = 0  # evict-engine round robin

        def evict_copy(dst, src):
            nonlocal ecnt
            if ecnt % 2 == 0:
                nc.vector.tensor_copy(dst, src)
            else:
                nc.scalar.copy(dst, src)
            ecnt += 1

        xT_r = xT_d.rearrange("(ci p) t -> p ci t", p=128)
        from contextlib import nullcontext
        if static_loop:
            loop_iter = [(nullcontext(iv), iv) for iv in range(0, tok, SB)]
        else:
            fc = tc.For_i(0, tok, SB,
                          hint_engines=(mybir.EngineType.PE,
                                        mybir.EngineType.DVE))
            loop_iter = [(fc, None)]
        for _ctx, _iv in loop_iter:
          with _ctx as _cv:
            iv = _iv if _iv is not None else _cv
            x_sb = xin.tile([128, 8, SB], WD)
            nc.sync.dma_start(out=x_sb, in_=xT_r[:, :, bass.ds(iv, SB)])

            # staging: T1 rows 64:128 = Q; T2 rows 64:128 = K, rows 0:64 = V
            T1 = stag.tile([128, NG, 16, 8], F32, name="T1")
            T2 = stag.tile([128, NG, 16, 8], F32, name="T2")

            # ---- qkv projection + scatter-evict (bias fused / on gpsimd) ----
            for co in range(24):
                psC1 = psA.tile([128, SB], F32)
                for ci in range(8):
                    nc.tensor.matmul(psC1, wq_sb[:, ci, co * 128:(co + 1) * 128],
                                     x_sb[:, ci, :], start=(ci == 0),
                                     stop=(ci == 7))
                kind, c = co // 8, co % 8
                for dlt in range(2):
                    src = psC1[64 * dlt:64 * dlt + 64, :].rearrange(
                        "p (g t) -> p g t", g=NG)
                    hslot = 2 * c + dlt
                    if kind == 0:
                        dst = T1[64:128, :, hslot, :]
                    elif kind == 1:
                        dst = T2[64:128, :, hslot, :]
                    else:
                        dst = T2[0:64, :, hslot, :]
                    bias = bcols_sb[64 * dlt:64 * dlt + 64, co:co + 1]
                    if dlt == 0:
                        # DVE evict with fused bias add
                        nc.vector.tensor_scalar_add(dst, src, bias)
                    elif GPS & 4:
                        # ACT plain evict, bias added SBUF-side on idle gpsimd
                        nc.scalar.copy(dst, src)
                        nc.gpsimd.tensor_scalar_add(dst, dst, bias)
                    else:
                        nc.vector.tensor_scalar_add(dst, src, bias)

            # ---- attention (8 batches of 4 groups) + out-proj per 128 tok ----
            for iss in range(2):
                S2 = s2p.tile([128, 8, SS], AD)
                for b4 in range(4 * iss, 4 * iss + 4):
                    psS = psSp.tile([128, 512], F32)
                    psV = psVp.tile([128, 4, 64], F32)
                    for j in range(4):
                        g = 4 * b4 + j
                        nc.tensor.matmul(psS[:, 128 * j:128 * j + 128],
                                         T2[64:128, g, :, :], T1[64:128, g, :, :],
                                         start=True, stop=True)
                        nc.tensor.transpose(psV[:, j, :], T2[0:64, g, :, :],
                                            idq[0:64, 0:64])
                    es4 = smx.tile([128, 512], F32)
                    nc.scalar.activation(es4, psS, Exp, scale=0.125)
                    if GPS & 2:
                        nc.gpsimd.tensor_mul(es4, es4, maskB_sb)
                    else:
                        nc.vector.tensor_mul(es4, es4, maskB_sb)
                    V8sb = smx.tile([128, 4, 66], F32)
                    nc.scalar.copy(V8sb[:, :, 0:64], psV)
                    if GPS & 1:
                        nc.gpsimd.memset(V8sb[:, :, 64:65], 1.0)
                    else:
                        nc.vector.memset(V8sb[:, :, 64:65], 1.0)
                    psC2 = psC2p.tile([128, 4, 66], F32)
                    for j in range(4):
                        nc.tensor.matmul(psC2[:, j, 0:65],
                                         es4[:, 128 * j:128 * j + 128],
                                         V8sb[:, j, 0:65], start=True, stop=True)
                    rec4 = smx.tile([128, 4], F32)
                    nc.vector.reciprocal(rec4, psC2[:, :, 64:65])
                    attno = smx.tile([128, 4, 64], AD)
                    for j in range(4):
                        if j % 2 == 0:
                            nc.scalar.activation(attno[:, j, :], psC2[:, j, 0:64],
                                                 Copy, scale=rec4[:, j:j + 1])
                        else:
                            nc.vector.tensor_scalar_mul(attno[:, j, :],
                                                        psC2[:, j, 0:64],
                                                        rec4[:, j:j + 1])
                    psT = psTp.tile([64, 4, 128], AD)
                    for j in range(4):
                        nc.tensor.transpose(psT[:, j, :], attno[:, j, :], idr)
                    # S2 pack: head slots 8*dlt..8*dlt+7 -> S2 rows 64*dlt+d
                    for dlt in range(2):
                        src = psT[:, :, 64 * dlt:64 * dlt + 64].rearrange(
                            "p j (h t) -> p j h t", h=8)
                        dst = S2[64 * dlt:64 * dlt + 64].rearrange(
                            "p c (gb gj t) -> p gj c gb t", gb=4, gj=4)[
                                :, :, :, b4 % 4, :]
                        evict_copy(dst, src)

                # ---- out-projection for this 128-token block ----
                outsb = outp.tile([128, C], F32)
                for nh in range(2):
                    psO = psOp.tile([128, 512], F32)
                    for c in range(8):
                        nc.tensor.matmul(psO, S2[:, c, :],
                                         wo_sb[:, c, 512 * nh:512 * nh + 512],
                                         start=(c == 0), stop=(c == 7))
                    nc.vector.tensor_add(outsb[:, 512 * nh:512 * nh + 512], psO,
                                         borep_sb[:, 512 * nh:512 * nh + 512])
                nc.sync.dma_start(out=out_d[bass.ds(iv + SS * iss, SS), :],
                                  in_=outsb)

    nc.compile()
    return nc


def build_pipe(tok, use_f32r=USE_F32R):
    """Software-pipelined build: attention of superblock k overlaps the
    projection of superblock k+1 inside one For_i body (2 superblocks per
    iteration, ping-pong staging halves packed into shared tiles)."""
    WD = F32R if use_f32r else F32
    AD = F32R if use_f32r else F32

    nc = bacc.Bacc("TRN2", target_bir_lowering=False, debug=False,
                   enable_asserts=True, num_devices=N_CORES)
    xT_d = nc.dram_tensor("xT", [C, tok], WD, kind="ExternalInput").ap()
    wqkvT_d = nc.dram_tensor("wqkvT", [C, 3 * C], WD, kind="ExternalInput").ap()
    woutT_d = nc.dram_tensor("woutT", [C, C], WD, kind="ExternalInput").ap()
    bcols_d = nc.dram_tensor("bcols", [128, 24], F32, kind="ExternalInput").ap()
    borep_d = nc.dram_tensor("borep", [128, C], F32, kind="ExternalInput").ap()
    maskB_d = nc.dram_tensor("maskB", [128, 512], F32, kind="ExternalInput").ap()
    out_d = nc.dram_tensor("out", [tok, C], F32, kind="ExternalOutput").ap()

    with tile.TileContext(nc) as tc, ExitStack() as ctx:
        consts = ctx.enter_context(tc.tile_pool(name="consts", bufs=1))
        xin = ctx.enter_context(tc.tile_pool(name="xin", bufs=1))
        stag = ctx.enter_context(tc.tile_pool(name="stag", bufs=1))
        smx = ctx.enter_context(tc.tile_pool(name="smx", bufs=1))
        s2p = ctx.enter_context(tc.tile_pool(name="s2p", bufs=1))
        outp = ctx.enter_context(tc.tile_pool(name="outp", bufs=1))
        psA = ctx.enter_context(tc.tile_pool(name="psA", bufs=2, space="PSUM"))
        psSp = ctx.enter_context(tc.tile_pool(name="psSp", bufs=2, space="PSUM"))
        psVp = ctx.enter_context(tc.tile_pool(name="psVp", bufs=1, space="PSUM"))
        psC2p = ctx.enter_context(tc.tile_pool(name="psC2p", bufs=1, space="PSUM"))
        psTp = ctx.enter_context(tc.tile_pool(name="psTp", bufs=1, space="PSUM"))
        psOp = ctx.enter_context(tc.tile_pool(name="psOp", bufs=1, space="PSUM"))

        wq_sb = consts.tile([128, 8, 3 * C], WD)
        nc.sync.dma_start(out=wq_sb, in_=wqkvT_d.rearrange("(ci p) f -> p ci f", p=128))
        wo_sb = consts.tile([128, 8, C], WD)
        nc.sync.dma_start(out=wo_sb, in_=woutT_d.rearrange("(ci p) f -> p ci f", p=128))
        bcols_sb = consts.tile([128, 24], F32)
        nc.sync.dma_start(out=bcols_sb, in_=bcols_d)
        borep_sb = consts.tile([128, C], F32)
        nc.sync.dma_start(out=borep_sb, in_=borep_d)
        maskB_sb = consts.tile([128, 512], F32)
        nc.sync.dma_start(out=maskB_sb, in_=maskB_d)
        idq = consts.tile([128, 128], F32)
        make_identity(nc, idq)
        if AD is F32:
            idr = idq
        else:
            idr = consts.tile([128, 128], AD)
            nc.vector.tensor_copy(idr, idq)

        # persistent ping-pong staging (half-set hb=0: Q/K upper, V lower)
        Q_AB = stag.tile([128, NG, 16, 8], F32, name="Q_AB")
        K_AB = stag.tile([128, NG, 16, 8], F32, name="K_AB")
        V_AB = stag.tile([128, NG, 16, 8], F32, name="V_AB")

        xT_r = xT_d.rearrange("(ci p) t -> p ci t", p=128)

        def emit_xload(piv):
            x_sb = xin.tile([128, 8, SB], WD)
            nc.sync.dma_start(out=x_sb, in_=xT_r[:, :, bass.ds(piv, SB)])
            return x_sb

        def qk_half(T, hb):
            return T[64 * (1 - hb):64 * (1 - hb) + 64]

        def v_half(hb):
            return V_AB[64 * hb:64 * hb + 64]

        def emit_proj_chunk(x_sb, co, hb):
            psC1 = psA.tile([128, SB], F32)
            for ci in range(8):
                nc.tensor.matmul(psC1, wq_sb[:, ci, co * 128:(co + 1) * 128],
                                 x_sb[:, ci, :], start=(ci == 0), stop=(ci == 7))
            kind, c = co // 8, co % 8
            for dlt in range(2):
                src = psC1[64 * dlt:64 * dlt + 64, :].rearrange(
                    "p (g t) -> p g t", g=NG)
                hslot = 2 * c + dlt
                if kind == 0:
                    dst = qk_half(Q_AB, hb)[:, :, hslot, :]
                elif kind == 1:
                    dst = qk_half(K_AB, hb)[:, :, hslot, :]
                else:
                    dst = v_half(hb)[:, :, hslot, :]
                bias = bcols_sb[64 * dlt:64 * dlt + 64, co:co + 1]
                if dlt == 0:
                    nc.vector.tensor_scalar_add(dst, src, bias)
                else:
                    nc.scalar.copy(dst, src)
                    nc.gpsimd.tensor_scalar_add(dst, dst, bias)

        def emit_attn_batch1(b4, hb):
            """scores + V transposes + exp + mask for groups 4*b4..4*b4+3."""
            psS = psSp.tile([128, 512], F32)
            psV = psVp.tile([128, 4, 64], F32)
            vb = 64 * hb
            for j in range(4):
                g = 4 * b4 + j
                nc.tensor.matmul(psS[:, 128 * j:128 * j + 128],
                                 qk_half(K_AB, hb)[:, g, :, :],
                                 qk_half(Q_AB, hb)[:, g, :, :],
                                 start=True, stop=True)
                nc.tensor.transpose(psV[:, j, :], v_half(hb)[:, g, :, :],
                                    idq[vb:vb + 64, vb:vb + 64])
            es4 = smx.tile([128, 512], F32)
            nc.scalar.activation(es4, psS, Exp, scale=0.125)
            nc.gpsimd.tensor_mul(es4, es4, maskB_sb)
            V8sb = smx.tile([128, 4, 66], F32)
            nc.scalar.copy(V8sb[:, :, 0:64], psV)
            nc.gpsimd.memset(V8sb[:, :, 64:65], 1.0)
            return es4, V8sb

        def emit_attn_batch2(b4, es4, V8sb, S2):
            psC2 = psC2p.tile([128, 4, 66], F32)
            for j in range(4):
                nc.tensor.matmul(psC2[:, j, 0:65], es4[:, 128 * j:128 * j + 128],
                                 V8sb[:, j, 0:65], start=True, stop=True)
            rec4 = smx.tile([128, 4], F32)
            nc.vector.reciprocal(rec4, psC2[:, :, 64:65])
            attno = smx.tile([128, 4, 64], AD)
            for j in range(4):
                if j % 2 == 0:
                    nc.scalar.activation(attno[:, j, :], psC2[:, j, 0:64],
                                         Copy, scale=rec4[:, j:j + 1])
                else:
                    nc.vector.tensor_scalar_mul(attno[:, j, :],
                                                psC2[:, j, 0:64],
                                                rec4[:, j:j + 1])
            psT = psTp.tile([64, 4, 128], AD)
            for j in range(4):
                nc.tensor.transpose(psT[:, j, :], attno[:, j, :], idr)
            for dlt in range(2):
                src = psT[:, :, 64 * dlt:64 * dlt + 64].rearrange(
                    "p j (h t) -> p j h t", h=8)
                dst = S2[64 * dlt:64 * dlt + 64].rearrange(
                    "p c (gb gj t) -> p gj c gb t", gb=4, gj=4)[:, :, :, b4 % 4, :]
                if dlt == 0:
                    nc.vector.tensor_copy(dst, src)
                else:
                    nc.scalar.copy(dst, src)

        def emit_outproj(S2, oiv, iss):
            outsb = outp.tile([128, C], F32)
            for nh in range(2):
                psO = psOp.tile([128, 512], F32)
                for c in range(8):
                    nc.tensor.matmul(psO, S2[:, c, :],
                                     wo_sb[:, c, 512 * nh:512 * nh + 512],
                                     start=(c == 0), stop=(c == 7))
                nc.vector.tensor_add(outsb[:, 512 * nh:512 * nh + 512], psO,
                                     borep_sb[:, 512 * nh:512 * nh + 512])
            nc.sync.dma_start(out=out_d[bass.ds(oiv + SS * iss, SS), :], in_=outsb)

        def emit_part(attn_oiv, attn_hb, proj_piv, proj_hb):
            """Weave attention of one superblock with projection of another.
            Either may be None (prologue/epilogue)."""
            x_sb = emit_xload(proj_piv) if proj_piv is not None else None
            S2 = None
            for b4 in range(8):
                if attn_oiv is not None:
                    if b4 % 4 == 0:
                        S2 = s2p.tile([128, 8, SS], AD, name="S2")
                    pend = emit_attn_batch1(b4, attn_hb)
                if x_sb is not None:
                    for co in range(3 * b4, 3 * b4 + 3):
                        emit_proj_chunk(x_sb, co, proj_hb)
                if attn_oiv is not None:
                    emit_attn_batch2(b4, *pend, S2)
                    if b4 % 4 == 3:
                        emit_outproj(S2, attn_oiv, b4 // 4)

        assert tok % (2 * SB) == 0 and tok >= 2 * SB
        emit_part(None, None, 0, 0)                      # prologue: proj sb0 -> A
        if tok > 2 * SB:
            with tc.For_i(0, tok - 2 * SB, 2 * SB,
                          hint_engines=(mybir.EngineType.PE, mybir.EngineType.DVE,
                                        mybir.EngineType.Activation)) as iv:
                emit_part(iv, 0, iv + SB, 1)             # attn A, proj -> B
                emit_part(iv + SB, 1, iv + 2 * SB, 0)    # attn B, proj -> A
        last = tok - 2 * SB
        emit_part(last, 0, tok - SB, 1)                  # attn A, proj last -> B
        emit_part(tok - SB, 1, None, None)               # attn B

    nc.compile()
    return nc


def _round_f32r(a):
    """Round fp32 to the f32r grid (drop 12 mantissa bits, round-to-nearest)."""
    b = np.ascontiguousarray(a, dtype=np.float32).view(np.uint32)
    b = ((b + (1 << 11)) >> 12) << 12
    return b.view(np.float32)


def _host_prep(x, w_qkv, b_qkv, w_out, b_out, use_f32r=USE_F32R):
    d = np.arange(D)
    perm_q = (192 * np.arange(H)[:, None] + d[None, :]).reshape(-1)
    perm = np.concatenate([perm_q, perm_q + 64, perm_q + 128])
    wqkvT = np.ascontiguousarray(w_qkv[perm, :].T, dtype=np.float32)
    bcols = np.ascontiguousarray(
        b_qkv[perm].reshape(24, 128).T, dtype=np.float32)
    # out-proj row perm: S2 row 128c+64dlt+d holds feature 64*(8dlt+c)+d
    co, dl = np.arange(8), np.arange(2)
    perm_o = (64 * (8 * dl[None, :, None] + co[:, None, None])
              + d[None, None, :]).reshape(-1)
    woutT = np.ascontiguousarray(w_out.T[perm_o, :], dtype=np.float32)
    borep = np.ascontiguousarray(
        np.broadcast_to(b_out[None, :], (128, C)), dtype=np.float32)
    maskB = np.tile((np.arange(128)[:, None] % 8
                     == np.arange(128)[None, :] % 8).astype(np.float32), (1, 4))
    xT = np.ascontiguousarray(x.T, dtype=np.float32)
    if use_f32r:
        xT = _round_f32r(xT)
        wqkvT = _round_f32r(wqkvT)
        woutT = _round_f32r(woutT)
    return xT, wqkvT, bcols, woutT, borep, maskB


_cache = {}


def kernel(x, w_qkv, b_qkv, w_out, b_out, _trace=False, _tmpdir=None):
    x = np.asarray(x)
    n = x.shape[0]
    tok = n // N_CORES
    xT, wqkvT, bcols, woutT, borep, maskB = _host_prep(
        np.asarray(x), np.asarray(w_qkv), np.asarray(b_qkv),
        np.asarray(w_out), np.asarray(b_out))
    pipe = os.environ.get("PIPE", "1") == "1"
    key = (tok, USE_F32R, pipe)
    if key not in _cache:
        _cache[key] = build_pipe(tok) if pipe else build(tok)
    nc = _cache[key]
    shared = dict(wqkvT=wqkvT, woutT=woutT, bcols=bcols, borep=borep, maskB=maskB)
    in_maps = [dict(xT=np.ascontiguousarray(xT[:, i * tok:(i + 1) * tok]), **shared)
               for i in range(N_CORES)]
    res = run_bass_kernel_spmd(nc, in_maps, core_ids=list(range(N_CORES)),
                               trace=_trace, tmpdir=_tmpdir)
    out = np.concatenate([res.results[i]["out"] for i in range(N_CORES)], axis=0)
    kernel.last_results = res
    return out



# revision 3
# speedup vs baseline: 11681.9560x; 11681.9560x over previous
"""Trainium2 Bass kernel for nn_MultiHeadAttention_72189810312078 (v2, fp16).

Computation (per token): qkv = x @ w_qkv.T + b_qkv; per-token attention over
the 16 heads with 16x16 score matrices; out = attn_out @ w_out.T + b_out.

Strategy: data-parallel over 8 NeuronCores (8192 tokens each). Everything on
the PE runs in float16 (10-bit mantissa, 1 cycle/row at any moving-dim size,
vs 4 cycles/row for f32/f32r below 256); PSUM accumulation stays fp32.

Per core, per 512-token superblock (SB=512, 64 groups of 8 tokens, 32
group-pairs):
  1) qkvT projection: 24 feature-chunk matmuls (N=512 moving), K=1024
     accumulated in PSUM over 8 chunk matmuls; bias fused into the eviction
     (DVE tensor_scalar_add / ACT activation bias).
  2) Evictions scatter into fp16 staging with groups split by parity s=g%2
     across partition halves:
       K2/V2 [128=(64s+d), gp, h, t];  Q2 [128, gp, s, h, t] with the
       complementary partition half zeroed once at startup.
  3) Scores for a group-PAIR in one K=128 matmul (zero-padding makes cross
     terms vanish): psS[(hk,tk),(s,hq,tq)] = K2[:,gp]^T @ Q2[:,gp]  (N=256).
     exp on ScalarE (scale=1/8) -> fp16, multiplicative block-diag mask on
     gpsimd.
  4) V^T for a pair in one PE transpose [128,128] (fp16 = 1 cyc/row);
     attnV per group with a ones column appended so the softmax denominator
     falls out of the same matmul; normalize via per-partition reciprocal
     scale on eviction.
  5) attn output pairs PE-transposed back to feature-major [128,128], packed
     into S2 [128=(dlt,d), chunk, token]; out-projection against
     host-permuted w_out.T (N=512 moving), bias added element-wise during
     eviction; result DMA'd row-major fp32.

Software pipeline: attention of superblock k overlaps the projection of
superblock k+1 inside one For_i body (2 superblocks per iteration,
ping-pong staging halves hb=0/1).
"""

import os
import sys
from contextlib import ExitStack, nullcontext

sys.path.insert(0, "/opt/trn_rl_repo")

import numpy as np

import concourse.bass as bass  # noqa: E402
import concourse.bacc as bacc  # noqa: E402
import concourse.tile as tile  # noqa: E402
from concourse import mybir  # noqa: E402
from concourse.bass_utils import run_bass_kernel_spmd  # noqa: E402
from concourse.masks import make_identity  # noqa: E402

F32 = mybir.dt.float32
F16 = mybir.dt.float16

N_CORES = 8
H, D, C = 16, 64, 1024
SB = 512          # tokens per superblock (projection moving dim)
SS = 128          # tokens per out-projection block
NG = SB // 8      # token groups per superblock (64)
NGP = NG // 2     # group pairs (32)
NB = NG // 4      # attention batches per superblock (16)

Exp = mybir.ActivationFunctionType.Exp
Copy = mybir.ActivationFunctionType.Copy
Ident = mybir.ActivationFunctionType.Identity


FLIP = os.environ.get("FLIP", "0") == "1"


def build(tok, repeat=1, flip=None):
    if flip is None:
        flip = FLIP
    nc = bacc.Bacc("TRN2", target_bir_lowering=False, debug=False,
                   enable_asserts=True, num_devices=N_CORES)
    xT_d = nc.dram_tensor("xT", [C, tok], F16, kind="ExternalInput").ap()
    wqkvT_d = nc.dram_tensor("wqkvT", [C, 3 * C], F16, kind="ExternalInput").ap()
    woutT_d = nc.dram_tensor("woutT", [C, C], F16, kind="ExternalInput").ap()
    bcols_d = nc.dram_tensor("bcols", [128, 24], F32, kind="ExternalInput").ap()
    borep_d = nc.dram_tensor("borep", [128, C], F32, kind="ExternalInput").ap()
    maskB_d = nc.dram_tensor("maskB", [128, 512], F16, kind="ExternalInput").ap()
    out_d = nc.dram_tensor("out", [tok, C], F32, kind="ExternalOutput").ap()

    with tile.TileContext(nc) as tc, ExitStack() as ctx:
        consts = ctx.enter_context(tc.tile_pool(name="consts", bufs=1))
        xin = ctx.enter_context(tc.tile_pool(name="xin", bufs=2))
        stag = ctx.enter_context(tc.tile_pool(name="stag", bufs=1))
        smx = ctx.enter_context(tc.tile_pool(name="smx", bufs=2))
        s2p = ctx.enter_context(tc.tile_pool(name="s2p", bufs=2))
        outp = ctx.enter_context(tc.tile_pool(name="outp", bufs=2))
        # PSUM: 8 banks total. psC1 x2, psS x2, psO x1, psV2t/psT shared
        # ring x2, psC2 x1.
        psA = ctx.enter_context(tc.tile_pool(name="psA", bufs=2, space="PSUM"))
        psSp = ctx.enter_context(tc.tile_pool(name="psSp", bufs=2,
                                              space="PSUM"))
        psVT = ctx.enter_context(
            tc.tile_pool(name="psVT", bufs=1 if flip else 2, space="PSUM"))
        psC2p = ctx.enter_context(
            tc.tile_pool(name="psC2p", bufs=2 if flip else 1, space="PSUM"))
        psOp = ctx.enter_context(tc.tile_pool(name="psOp", bufs=1, space="PSUM"))

        # ---- constants ----
        wq_sb = consts.tile([128, 8, 3 * C], F16)
        nc.sync.dma_start(out=wq_sb, in_=wqkvT_d.rearrange("(ci p) f -> p ci f", p=128))
        wo_sb = consts.tile([128, 8, C], F16)
        nc.sync.dma_start(out=wo_sb, in_=woutT_d.rearrange("(ci p) f -> p ci f", p=128))
        bcols_sb = consts.tile([128, 24], F32)
        nc.sync.dma_start(out=bcols_sb, in_=bcols_d)
        borep_sb = consts.tile([128, C], F32)
        nc.sync.dma_start(out=borep_sb, in_=borep_d)
        maskB_sb = consts.tile([128, 2, 256], F16)
        nc.sync.dma_start(out=maskB_sb,
                          in_=maskB_d.rearrange("p (i x) -> p i x", i=2))
        idq = consts.tile([128, 128], F32)
        make_identity(nc, idq)
        idf = consts.tile([128, 128], F16)
        nc.vector.tensor_copy(idf, idq)

        # persistent fp16 staging, ping-pong on hb
        Q2 = stag.tile([128, 2, NGP, 2, H, 8], F16, name="Q2")
        K2 = stag.tile([128, 2, NGP, H, 8], F16, name="K2")
        V2 = stag.tile([128, 2, NGP, H, 8], F16, name="V2")
        # zero the complementary partition half of Q2 once; evictions never
        # touch it, so the pair-matmul cross terms stay exactly zero
        for s in range(2):
            z0 = 64 * (1 - s)
            nc.vector.memset(Q2[z0:z0 + 64, :, :, s, :, :], 0.0)

        # proj evictions alternate DVE/ACT (Pool cannot read PSUM)
        PROJ_ENGS = ("v", "a", "v", "a", "v", "a", "v", "a")

        xT_r = xT_d.rearrange("(ci p) t -> p ci t", p=128)

        def emit_xload(piv):
            x_sb = xin.tile([128, 8, SB], F16)
            nc.sync.dma_start(out=x_sb, in_=xT_r[:, :, bass.ds(piv, SB)])
            return x_sb

        def emit_proj_chunk(x_sb, co, hb):
            psC1 = psA.tile([128, SB], F32, name="psC1")
            for ci in range(8):
                nc.tensor.matmul(psC1, wq_sb[:, ci, co * 128:(co + 1) * 128],
                                 x_sb[:, ci, :], start=(ci == 0), stop=(ci == 7))
            kind, c = co // 8, co % 8
            srcg = psC1.rearrange("p (gp s t) -> p gp s t", gp=NGP, s=2)
            for dlt in range(2):
                h = 2 * c + dlt
                bias = bcols_sb[64 * dlt:64 * dlt + 64, co:co + 1]
                for s in range(2):
                    src = srcg[64 * dlt:64 * dlt + 64, :, s, :]
                    if kind == 0:
                        dst = Q2[64 * s:64 * s + 64, hb, :, s, h, :]
                    elif kind == 1:
                        dst = K2[64 * s:64 * s + 64, hb, :, h, :]
                    else:
                        dst = V2[64 * s:64 * s + 64, hb, :, h, :]
                    if PROJ_ENGS[(co % 2) * 4 + 2 * dlt + s] == "v":
                        nc.vector.tensor_scalar_add(dst, src, bias)
                    else:
                        nc.scalar.activation(dst, src, Ident, bias=bias)

        def emit_attn_batch1(b4, hb):
            """scores (paired) + V^T (paired) + exp + mask for groups
            4*b4 .. 4*b4+3."""
            psS = psSp.tile([128, 2, 256], F32, name="psS")
            psV2t = psVT.tile([128, 2, 128], F16, name="psV2t", tag="vt")
            for i in range(2):
                gp = 2 * b4 + i
                nc.tensor.matmul(psS[:, i, :], K2[:, hb, gp, :, :],
                                 Q2[:, hb, gp, :, :, :], start=True, stop=True)
                nc.tensor.transpose(psV2t[:, i, :], V2[:, hb, gp, :, :], idf)
            es4 = smx.tile([128, 2, 256], F16)
            nc.scalar.activation(es4, psS, Exp, scale=0.125)
            nc.gpsimd.tensor_mul(es4, es4, maskB_sb)
            V8sb = smx.tile([128, 2, 2, 66], F16)
            nc.vector.tensor_copy(V8sb[:, :, :, 0:64],
                                  psV2t.rearrange("p i (s d) -> p i s d", s=2))
            nc.gpsimd.memset(V8sb[:, :, :, 64:65], 1.0)
            return es4, V8sb

        def emit_attn_batch2_flip(b4, es4, V8sb, S2):
            """Flipped attnV: lhsT = V8 (65 weight columns — cheap ldweights),
            rhs = es4 moving. psC2 [65, g, (hq,t)] is feature-major; row 64 =
            softmax denominators. Normalize fuses into the S2 eviction with a
            Pool-broadcast (SBUF->SBUF) reciprocal row."""
            es4r = es4.rearrange("p i (s x) -> p i s x", s=2)
            psC2 = psC2p.tile([65, 4, 128], F32, name="psC2f")
            for j in range(4):
                i, s = j // 2, j % 2
                nc.tensor.matmul(psC2[:, j, :], V8sb[:, i, s, 0:65],
                                 es4r[:, i, s, :], start=True, stop=True)
            rec = smx.tile([1, 4, 128], F32)
            nc.vector.reciprocal(rec, psC2[64:65, :, :])
            recb = smx.tile([64, 4, 128], F32)
            nc.gpsimd.partition_broadcast(recb, rec)
            S2r = S2.rearrange("p c (gq t) -> p c gq t", gq=16)
            bb = b4 % 4
            for dlt in range(2):
                cols = slice(64 * dlt, 64 * dlt + 64)
                src = psC2[0:64, :, cols].rearrange("p g (c t) -> p c g t", c=8)
                scl = recb[:, :, cols].rearrange("p g (c t) -> p c g t", c=8)
                dst = S2r[64 * dlt:64 * dlt + 64, :, 4 * bb:4 * bb + 4, :]
                nc.vector.tensor_mul(dst, src, scl)

        s2cnt = 0

        def s2_evict(dst, src):
            nonlocal s2cnt
            if s2cnt % 2 == 0:
                nc.vector.tensor_copy(dst, src)
            else:
                nc.scalar.copy(dst, src)
            s2cnt += 1

        def emit_attn_batch2_noflip(b4, es4, V8sb, S2):
            es4r = es4.rearrange("p i (s x) -> p i s x", s=2)
            psC2 = psC2p.tile([128, 4, 66], F32, name="psC2n")
            for j in range(4):
                i, s = j // 2, j % 2
                nc.tensor.matmul(psC2[:, j, 0:65], es4r[:, i, s, :],
                                 V8sb[:, i, s, 0:65], start=True, stop=True)
            rec4 = smx.tile([128, 4, 1], F32)
            nc.vector.reciprocal(rec4, psC2[:, :, 64:65])
            attno = smx.tile([128, 4, 64], F16)
            nsrc, recb4 = bass.broadcast_tensor_aps(psC2[:, :, 0:64], rec4)
            nc.vector.tensor_mul(attno, nsrc, recb4)
            psT = psVT.tile([128, 2, 128], F16, name="psT", tag="vt")
            for jj in range(2):
                nc.tensor.transpose(psT[:, jj, :],
                                    attno[:, 2 * jj:2 * jj + 2, :], idf)
            S2r = S2.rearrange("p c (gq t) -> p c gq t", gq=16)
            psTr = psT.rearrange("p jj (c t) -> p jj c t", c=16)
            for jx in range(2):
                for dlt in range(2):
                    src = psTr[64 * jx:64 * jx + 64, :,
                               8 * dlt:8 * dlt + 8, :].rearrange(
                                   "p jj c t -> p c jj t")
                    bb = b4 % 4
                    dst = S2r[64 * dlt:64 * dlt + 64, :, :, :].rearrange(
                        "p c (ga jj gb) t -> p c ga jj gb t",
                        jj=2, gb=2)[:, :, bb, :, jx, :]
                    s2_evict(dst, src)

        emit_attn_batch2 = (emit_attn_batch2_flip if flip
                            else emit_attn_batch2_noflip)

        def emit_outproj(S2, oiv, ssi):
            outsb = outp.tile([128, C], F32)
            for nh in range(2):
                psO = psOp.tile([128, 512], F32, name="psO")
                for c in range(8):
                    nc.tensor.matmul(psO, S2[:, c, :],
                                     wo_sb[:, c, 512 * nh:512 * nh + 512],
                                     start=(c == 0), stop=(c == 7))
                nc.vector.tensor_add(outsb[:, 512 * nh:512 * nh + 512], psO,
                                     borep_sb[:, 512 * nh:512 * nh + 512])
            nc.sync.dma_start(out=out_d[bass.ds(oiv + SS * ssi, SS), :],
                              in_=outsb)

        def emit_part(attn_oiv, attn_hb, proj_piv, proj_hb):
            """Weave attention of one superblock with projection of another.
            Either may be None (prologue/epilogue)."""
            x_sb = emit_xload(proj_piv) if proj_piv is not None else None
            S2 = None
            for b4 in range(NB):
                if attn_oiv is not None:
                    if b4 % 4 == 0:
                        S2 = s2p.tile([128, 8, SS], F16, name="S2")
                    pend = emit_attn_batch1(b4, attn_hb)
                if x_sb is not None:
                    for co in range((3 * b4) // 2, (3 * (b4 + 1)) // 2):
                        emit_proj_chunk(x_sb, co, proj_hb)
                if attn_oiv is not None:
                    emit_attn_batch2(b4, *pend, S2)
                    if b4 % 4 == 3:
                        emit_outproj(S2, attn_oiv, b4 // 4)

        assert tok % (2 * SB) == 0 and tok >= 2 * SB

        def emit_whole():
            # U superblocks per For_i body; sb k uses staging half k%2
            U = 2 * int(os.environ.get("UNROLL", "2"))
            if os.environ.get("STATIC", "0") == "1":
                U = tok // SB  # fully static: no hardware loop
            nsb = tok // SB
            emit_part(None, None, 0, 0)                  # prologue: proj sb0 -> A
            nloop = ((nsb - 1) // U) * U                 # sbs covered by loop
            if nloop > 0:
                with tc.For_i(0, nloop * SB, U * SB,
                              hint_engines=(mybir.EngineType.PE,
                                            mybir.EngineType.DVE,
                                            mybir.EngineType.Activation)
                              ) as iv:
                    for j in range(U):
                        emit_part(iv + j * SB, j % 2,
                                  iv + (j + 1) * SB, (j + 1) % 2)
            for k in range(nloop, nsb - 1):
                emit_part(k * SB, k % 2, (k + 1) * SB, (k + 1) % 2)
            emit_part((nsb - 1) * SB, (nsb - 1) % 2, None, None)

        if repeat == 1:
            emit_whole()
        else:
            # bench-only: run the whole kernel `repeat` times on-device so
            # steady-state HW time can be differenced out of RPC overhead
            with tc.For_i(0, repeat, 1):
                emit_whole()

    nc.compile()
    return nc


def _host_prep(x, w_qkv, b_qkv, w_out, b_out):
    d = np.arange(D)
    perm_q = (192 * np.arange(H)[:, None] + d[None, :]).reshape(-1)
    perm = np.concatenate([perm_q, perm_q + 64, perm_q + 128])
    wqkvT = np.ascontiguousarray(w_qkv[perm, :].T, dtype=np.float16)
    bcols = np.ascontiguousarray(
        b_qkv[perm].reshape(24, 128).T, dtype=np.float32)
    # out-proj row perm: S2 row 128c+64dlt+d holds feature 64*(8dlt+c)+d
    co, dl = np.arange(8), np.arange(2)
    perm_o = (64 * (8 * dl[None, :, None] + co[:, None, None])
              + d[None, None, :]).reshape(-1)
    woutT = np.ascontiguousarray(w_out.T[perm_o, :], dtype=np.float16)
    borep = np.ascontiguousarray(
        np.broadcast_to(b_out[None, :], (128, C)), dtype=np.float32)
    maskB = np.tile((np.arange(128)[:, None] % 8
                     == np.arange(128)[None, :] % 8).astype(np.float16), (1, 4))
    xT = np.ascontiguousarray(x.T, dtype=np.float16)
    return xT, wqkvT, bcols, woutT, borep, maskB


_cache = {}


def kernel(x, w_qkv, b_qkv, w_out, b_out, _trace=False, _tmpdir=None):
    x = np.asarray(x)
    n = x.shape[0]
    tok = n // N_CORES
    xT, wqkvT, bcols, woutT, borep, maskB = _host_prep(
        np.asarray(x), np.asarray(w_qkv), np.asarray(b_qkv),
        np.asarray(w_out), np.asarray(b_out))
    if tok not in _cache:
        _cache[tok] = build(tok)
    nc = _cache[tok]
    shared = dict(wqkvT=wqkvT, woutT=woutT, bcols=bcols, borep=borep, maskB=maskB)
    in_maps = [dict(xT=np.ascontiguousarray(xT[:, i * tok:(i + 1) * tok]), **shared)
               for i in range(N_CORES)]
    res = run_bass_kernel_spmd(nc, in_maps, core_ids=list(range(N_CORES)),
                               trace=_trace, tmpdir=_tmpdir)
    out = np.concatenate([res.results[i]["out"] for i in range(N_CORES)], axis=0)
    kernel.last_results = res
    return out


# revision 6
# speedup vs baseline: 12315.6505x; 1.0542x over previous
"""Trainium2 Bass kernel for nn_MultiHeadAttention_72189810312078 (v2, fp16).

Computation (per token): qkv = x @ w_qkv.T + b_qkv; per-token attention over
the 16 heads with 16x16 score matrices; out = attn_out @ w_out.T + b_out.

Strategy: data-parallel over 8 NeuronCores (8192 tokens each). Everything on
the PE runs in float16 (10-bit mantissa, 1 cycle/row at any moving-dim size,
vs 4 cycles/row for f32/f32r below 256); PSUM accumulation stays fp32.

Per core, per 512-token superblock (SB=512, 64 groups of 8 tokens, 32
group-pairs):
  1) qkvT projection: 24 feature-chunk matmuls (N=512 moving), K=1024
     accumulated in PSUM over 8 chunk matmuls; bias fused into the eviction
     (DVE tensor_scalar_add / ACT activation bias).
  2) Evictions scatter into fp16 staging with groups split by parity s=g%2
     across partition halves:
       K2/V2 [128=(64s+d), gp, h, t];  Q2 [128, gp, s, h, t] with the
       complementary partition half zeroed once at startup.
  3) Scores for a group-PAIR in one K=128 matmul (zero-padding makes cross
     terms vanish): psS[(hk,tk),(s,hq,tq)] = K2[:,gp]^T @ Q2[:,gp]  (N=256).
     exp on ScalarE (scale=1/8) -> fp16, multiplicative block-diag mask on
     gpsimd.
  4) V^T for a pair in one PE transpose [128,128] (fp16 = 1 cyc/row);
     attnV per group (lhsT = es4) with a ones column appended to V8 so the
     softmax denominator falls out of the same matmul; normalization is one
     DVE multiply with the reciprocal broadcast along the free dim.
  5) attn output pairs PE-transposed back to feature-major [128,128], packed
     into S2 [128=(dlt,d), chunk, token]; out-projection against
     host-permuted w_out.T (N=512 moving), bias added element-wise during
     eviction; result DMA'd row-major fp32.

Software pipeline: attention of superblock k overlaps the projection of
superblock k+1; the For_i body covers 2*UNROLL superblocks (default 8 —
fewer loop boundaries measurably help), staging ping-pongs on hb = sb%2. Evictions alternate DVE/ACT (GPSIMD cannot touch
PSUM on TRN2); the block-diag mask multiply runs on the otherwise-idle
GPSIMD; PSUM is exactly 8 banks: psC1 x2, psS x2, psV2t/psT ring x2,
psC2 x1, psO x1.
"""

import os
import sys
from contextlib import ExitStack, nullcontext

sys.path.insert(0, "/opt/trn_rl_repo")

import numpy as np

import concourse.bass as bass  # noqa: E402
import concourse.bacc as bacc  # noqa: E402
import concourse.tile as tile  # noqa: E402
from concourse import mybir  # noqa: E402
from concourse.bass_utils import run_bass_kernel_spmd  # noqa: E402
from concourse.masks import make_identity  # noqa: E402

F32 = mybir.dt.float32
F16 = mybir.dt.float16

N_CORES = 8
H, D, C = 16, 64, 1024
SB = 512          # tokens per superblock (projection moving dim)
SS = 128          # tokens per out-projection block
NG = SB // 8      # token groups per superblock (64)
NGP = NG // 2     # group pairs (32)
NB = NG // 4      # attention batches per superblock (16)

Exp = mybir.ActivationFunctionType.Exp
Copy = mybir.ActivationFunctionType.Copy
Ident = mybir.ActivationFunctionType.Identity


FLIP = os.environ.get("FLIP", "0") == "1"


def build(tok, repeat=1, flip=None):
    if flip is None:
        flip = FLIP
    nc = bacc.Bacc("TRN2", target_bir_lowering=False, debug=False,
                   enable_asserts=True, num_devices=N_CORES)
    xT_d = nc.dram_tensor("xT", [C, tok], F16, kind="ExternalInput").ap()
    wqkvT_d = nc.dram_tensor("wqkvT", [C, 3 * C], F16, kind="ExternalInput").ap()
    woutT_d = nc.dram_tensor("woutT", [C, C], F16, kind="ExternalInput").ap()
    bcols_d = nc.dram_tensor("bcols", [128, 24], F32, kind="ExternalInput").ap()
    borep_d = nc.dram_tensor("borep", [128, C], F32, kind="ExternalInput").ap()
    maskB_d = nc.dram_tensor("maskB", [128, 512], F16, kind="ExternalInput").ap()
    out_d = nc.dram_tensor("out", [tok, C], F32, kind="ExternalOutput").ap()

    with tile.TileContext(nc) as tc, ExitStack() as ctx:
        consts = ctx.enter_context(tc.tile_pool(name="consts", bufs=1))
        xin = ctx.enter_context(tc.tile_pool(name="xin", bufs=2))
        stag = ctx.enter_context(tc.tile_pool(name="stag", bufs=1))
        smx = ctx.enter_context(tc.tile_pool(name="smx", bufs=2))
        s2p = ctx.enter_context(tc.tile_pool(name="s2p", bufs=2))
        outp = ctx.enter_context(tc.tile_pool(name="outp", bufs=2))
        # PSUM: 8 banks total. psC1 x2, psS x2, psO x1, psV2t/psT shared
        # ring x2, psC2 x1.
        psA = ctx.enter_context(tc.tile_pool(name="psA", bufs=2, space="PSUM"))
        psSp = ctx.enter_context(tc.tile_pool(name="psSp", bufs=2,
                                              space="PSUM"))
        psVT = ctx.enter_context(
            tc.tile_pool(name="psVT", bufs=1 if flip else 2, space="PSUM"))
        psC2p = ctx.enter_context(
            tc.tile_pool(name="psC2p", bufs=2 if flip else 1, space="PSUM"))
        psOp = ctx.enter_context(tc.tile_pool(name="psOp", bufs=1, space="PSUM"))

        # ---- constants ----
        wq_sb = consts.tile([128, 8, 3 * C], F16)
        nc.sync.dma_start(out=wq_sb, in_=wqkvT_d.rearrange("(ci p) f -> p ci f", p=128))
        wo_sb = consts.tile([128, 8, C], F16)
        nc.sync.dma_start(out=wo_sb, in_=woutT_d.rearrange("(ci p) f -> p ci f", p=128))
        bcols_sb = consts.tile([128, 24], F32)
        nc.sync.dma_start(out=bcols_sb, in_=bcols_d)
        borep_sb = consts.tile([128, C], F32)
        nc.sync.dma_start(out=borep_sb, in_=borep_d)
        maskB_sb = consts.tile([128, 2, 256], F16)
        nc.sync.dma_start(out=maskB_sb,
                          in_=maskB_d.rearrange("p (i x) -> p i x", i=2))
        idq = consts.tile([128, 128], F32)
        make_identity(nc, idq)
        idf = consts.tile([128, 128], F16)
        nc.vector.tensor_copy(idf, idq)

        # persistent fp16 staging, ping-pong on hb
        Q2 = stag.tile([128, 2, NGP, 2, H, 8], F16, name="Q2")
        K2 = stag.tile([128, 2, NGP, H, 8], F16, name="K2")
        V2 = stag.tile([128, 2, NGP, H, 8], F16, name="V2")
        # zero the complementary partition half of Q2 once; evictions never
        # touch it, so the pair-matmul cross terms stay exactly zero
        for s in range(2):
            z0 = 64 * (1 - s)
            nc.vector.memset(Q2[z0:z0 + 64, :, :, s, :, :], 0.0)

        # proj evictions alternate DVE/ACT (Pool cannot read PSUM)
        PROJ_ENGS = ("v", "a", "v", "a", "v", "a", "v", "a")

        xT_r = xT_d.rearrange("(ci p) t -> p ci t", p=128)

        def emit_xload(piv):
            x_sb = xin.tile([128, 8, SB], F16)
            nc.sync.dma_start(out=x_sb, in_=xT_r[:, :, bass.ds(piv, SB)])
            return x_sb

        def emit_proj_chunk(x_sb, co, hb):
            psC1 = psA.tile([128, SB], F32, name="psC1")
            for ci in range(8):
                nc.tensor.matmul(psC1, wq_sb[:, ci, co * 128:(co + 1) * 128],
                                 x_sb[:, ci, :], start=(ci == 0), stop=(ci == 7))
            kind, c = co // 8, co % 8
            srcg = psC1.rearrange("p (gp s t) -> p gp s t", gp=NGP, s=2)
            for dlt in range(2):
                h = 2 * c + dlt
                bias = bcols_sb[64 * dlt:64 * dlt + 64, co:co + 1]
                for s in range(2):
                    src = srcg[64 * dlt:64 * dlt + 64, :, s, :]
                    if kind == 0:
                        dst = Q2[64 * s:64 * s + 64, hb, :, s, h, :]
                    elif kind == 1:
                        dst = K2[64 * s:64 * s + 64, hb, :, h, :]
                    else:
                        dst = V2[64 * s:64 * s + 64, hb, :, h, :]
                    if PROJ_ENGS[(co % 2) * 4 + 2 * dlt + s] == "v":
                        nc.vector.tensor_scalar_add(dst, src, bias)
                    else:
                        nc.scalar.activation(dst, src, Ident, bias=bias)

        def emit_attn_batch1(b4, hb):
            """scores (paired) + V^T (paired) + exp + mask for groups
            4*b4 .. 4*b4+3."""
            psS = psSp.tile([128, 2, 256], F32, name="psS")
            psV2t = psVT.tile([128, 2, 128], F16, name="psV2t", tag="vt")
            for i in range(2):
                gp = 2 * b4 + i
                nc.tensor.matmul(psS[:, i, :], K2[:, hb, gp, :, :],
                                 Q2[:, hb, gp, :, :, :], start=True, stop=True)
                nc.tensor.transpose(psV2t[:, i, :], V2[:, hb, gp, :, :], idf)
            es4 = smx.tile([128, 2, 256], F16)
            nc.scalar.activation(es4, psS, Exp, scale=0.125)
            nc.gpsimd.tensor_mul(es4, es4, maskB_sb)
            V8sb = smx.tile([128, 2, 2, 66], F16)
            nc.vector.tensor_copy(V8sb[:, :, :, 0:64],
                                  psV2t.rearrange("p i (s d) -> p i s d", s=2))
            nc.gpsimd.memset(V8sb[:, :, :, 64:65], 1.0)
            return es4, V8sb

        def emit_attn_batch2_flip(b4, es4, V8sb, S2):
            """Flipped attnV: lhsT = V8 (65 weight columns — cheap ldweights),
            rhs = es4 moving. psC2 [65, g, (hq,t)] is feature-major; row 64 =
            softmax denominators. Normalize fuses into the S2 eviction with a
            Pool-broadcast (SBUF->SBUF) reciprocal row."""
            es4r = es4.rearrange("p i (s x) -> p i s x", s=2)
            psC2 = psC2p.tile([65, 4, 128], F32, name="psC2f")
            for j in range(4):
                i, s = j // 2, j % 2
                nc.tensor.matmul(psC2[:, j, :], V8sb[:, i, s, 0:65],
                                 es4r[:, i, s, :], start=True, stop=True)
            rec = smx.tile([1, 4, 128], F32)
            nc.vector.reciprocal(rec, psC2[64:65, :, :])
            recb = smx.tile([64, 4, 128], F32)
            nc.gpsimd.partition_broadcast(recb, rec)
            S2r = S2.rearrange("p c (gq t) -> p c gq t", gq=16)
            bb = b4 % 4
            for dlt in range(2):
                cols = slice(64 * dlt, 64 * dlt + 64)
                src = psC2[0:64, :, cols].rearrange("p g (c t) -> p c g t", c=8)
                scl = recb[:, :, cols].rearrange("p g (c t) -> p c g t", c=8)
                dst = S2r[64 * dlt:64 * dlt + 64, :, 4 * bb:4 * bb + 4, :]
                nc.vector.tensor_mul(dst, src, scl)

        s2cnt = 0

        def s2_evict(dst, src):
            nonlocal s2cnt
            if s2cnt % 2 == 0:
                nc.vector.tensor_copy(dst, src)
            else:
                nc.scalar.copy(dst, src)
            s2cnt += 1

        def emit_attn_batch2_noflip(b4, es4, V8sb, S2):
            es4r = es4.rearrange("p i (s x) -> p i s x", s=2)
            psC2 = psC2p.tile([128, 4, 66], F32, name="psC2n")
            for j in range(4):
                i, s = j // 2, j % 2
                nc.tensor.matmul(psC2[:, j, 0:65], es4r[:, i, s, :],
                                 V8sb[:, i, s, 0:65], start=True, stop=True)
            rec4 = smx.tile([128, 4, 1], F32)
            nc.vector.reciprocal(rec4, psC2[:, :, 64:65])
            attno = smx.tile([128, 4, 64], F16)
            nsrc, recb4 = bass.broadcast_tensor_aps(psC2[:, :, 0:64], rec4)
            nc.vector.tensor_mul(attno, nsrc, recb4)
            psT = psVT.tile([128, 2, 128], F16, name="psT", tag="vt")
            for jj in range(2):
                nc.tensor.transpose(psT[:, jj, :],
                                    attno[:, 2 * jj:2 * jj + 2, :], idf)
            S2r = S2.rearrange("p c (gq t) -> p c gq t", gq=16)
            psTr = psT.rearrange("p jj (c t) -> p jj c t", c=16)
            for jx in range(2):
                for dlt in range(2):
                    src = psTr[64 * jx:64 * jx + 64, :,
                               8 * dlt:8 * dlt + 8, :].rearrange(
                                   "p jj c t -> p c jj t")
                    bb = b4 % 4
                    dst = S2r[64 * dlt:64 * dlt + 64, :, :, :].rearrange(
                        "p c (ga jj gb) t -> p c ga jj gb t",
                        jj=2, gb=2)[:, :, bb, :, jx, :]
                    s2_evict(dst, src)

        emit_attn_batch2 = (emit_attn_batch2_flip if flip
                            else emit_attn_batch2_noflip)

        def emit_outproj(S2, oiv, ssi):
            outsb = outp.tile([128, C], F32)
            for nh in range(2):
                psO = psOp.tile([128, 512], F32, name="psO")
                for c in range(8):
                    nc.tensor.matmul(psO, S2[:, c, :],
                                     wo_sb[:, c, 512 * nh:512 * nh + 512],
                                     start=(c == 0), stop=(c == 7))
                nc.vector.tensor_add(outsb[:, 512 * nh:512 * nh + 512], psO,
                                     borep_sb[:, 512 * nh:512 * nh + 512])
            nc.sync.dma_start(out=out_d[bass.ds(oiv + SS * ssi, SS), :],
                              in_=outsb)

        def emit_part(attn_oiv, attn_hb, proj_piv, proj_hb):
            """Weave attention of one superblock with projection of another.
            Either may be None (prologue/epilogue)."""
            x_sb = emit_xload(proj_piv) if proj_piv is not None else None
            S2 = None
            for b4 in range(NB):
                if attn_oiv is not None:
                    if b4 % 4 == 0:
                        S2 = s2p.tile([128, 8, SS], F16, name="S2")
                    pend = emit_attn_batch1(b4, attn_hb)
                if x_sb is not None:
                    for co in range((3 * b4) // 2, (3 * (b4 + 1)) // 2):
                        emit_proj_chunk(x_sb, co, proj_hb)
                if attn_oiv is not None:
                    emit_attn_batch2(b4, *pend, S2)
                    if b4 % 4 == 3:
                        emit_outproj(S2, attn_oiv, b4 // 4)

        assert tok % (2 * SB) == 0 and tok >= 2 * SB

        def emit_whole():
            # U superblocks per For_i body; sb k uses staging half k%2
            U = 2 * int(os.environ.get("UNROLL", "4"))
            if os.environ.get("STATIC", "0") == "1":
                U = tok // SB  # fully static: no hardware loop
            nsb = tok // SB
            emit_part(None, None, 0, 0)                  # prologue: proj sb0 -> A
            nloop = ((nsb - 1) // U) * U                 # sbs covered by loop
            if nloop > 0:
                with tc.For_i(0, nloop * SB, U * SB,
                              hint_engines=(mybir.EngineType.PE,
                                            mybir.EngineType.DVE,
                                            mybir.EngineType.Activation)
                              ) as iv:
                    for j in range(U):
                        emit_part(iv + j * SB, j % 2,
                                  iv + (j + 1) * SB, (j + 1) % 2)
            for k in range(nloop, nsb - 1):
                emit_part(k * SB, k % 2, (k + 1) * SB, (k + 1) % 2)
            emit_part((nsb - 1) * SB, (nsb - 1) % 2, None, None)

        if repeat == 1:
            emit_whole()
        else:
            # bench-only: run the whole kernel `repeat` times on-device so
            # steady-state HW time can be differenced out of RPC overhead
            with tc.For_i(0, repeat, 1):
                emit_whole()

    nc.compile()
    return nc


def _host_prep(x, w_qkv, b_qkv, w_out, b_out):
    d = np.arange(D)
    perm_q = (192 * np.arange(H)[:, None] + d[None, :]).reshape(-1)
    perm = np.concatenate([perm_q, perm_q + 64, perm_q + 128])
    wqkvT = np.ascontiguousarray(w_qkv[perm, :].T, dtype=np.float16)
    bcols = np.ascontiguousarray(
        b_qkv[perm].reshape(24, 128).T, dtype=np.float32)
    # out-proj row perm: S2 row 128c+64dlt+d holds feature 64*(8dlt+c)+d
    co, dl = np.arange(8), np.arange(2)
    perm_o = (64 * (8 * dl[None, :, None] + co[:, None, None])
              + d[None, None, :]).reshape(-1)
    woutT = np.ascontiguousarray(w_out.T[perm_o, :], dtype=np.float16)
    borep = np.ascontiguousarray(
        np.broadcast_to(b_out[None, :], (128, C)), dtype=np.float32)
    maskB = np.tile((np.arange(128)[:, None] % 8
                     == np.arange(128)[None, :] % 8).astype(np.float16), (1, 4))
    xT = np.ascontiguousarray(x.T, dtype=np.float16)
    return xT, wqkvT, bcols, woutT, borep, maskB


_cache = {}


def kernel(x, w_qkv, b_qkv, w_out, b_out, _trace=False, _tmpdir=None):
    x = np.asarray(x)
    n = x.shape[0]
    tok = n // N_CORES
    xT, wqkvT, bcols, woutT, borep, maskB = _host_prep(
        np.asarray(x), np.asarray(w_qkv), np.asarray(b_qkv),
        np.asarray(w_out), np.asarray(b_out))
    if tok not in _cache:
        _cache[tok] = build(tok)
    nc = _cache[tok]
    shared = dict(wqkvT=wqkvT, woutT=woutT, bcols=bcols, borep=borep, maskB=maskB)
    in_maps = [dict(xT=np.ascontiguousarray(xT[:, i * tok:(i + 1) * tok]), **shared)
               for i in range(N_CORES)]
    res = run_bass_kernel_spmd(nc, in_maps, core_ids=list(range(N_CORES)),
                               trace=_trace, tmpdir=_tmpdir)
    out = np.concatenate([res.results[i]["out"] for i in range(N_CORES)], axis=0)
    kernel.last_results = res
    return out
